# revision 32
# baseline (speedup 1.0000x reference)
"""Trainium2 Bass kernel for nn_Attention_24043226923261.

Per-pixel cross-attention: RMSNorm(c) -> kv proj -> softmax over N=8 context
slices with a query shared across the 32x32 spatial grid -> out proj.

Sharding: data-parallel over B=8 across the 8 NeuronCores (core b owns batch
b). Zero collectives.

Host-side weight folding (exact math):
  - query path qh = silu(emb[q]@w1+b1)@w2+b2 is a [8,512] tensor; dots =
    qh . (c_norm @ w_k) = c_norm @ (w_k @ qh^T), so fold qh, attn_scale and
    rms_w into a per-core [256,8] matrix wq.  k is never materialized and the
    kv projection halves to v-only.
  - rms_w folds into wv/wq; the per-token rsqrt(mean(c^2)) scale s_n[t] is
    applied on device: on the k side inside exp(), on the v side folded into
    the softmax weights.

Device-side structure (v2):
  - fp8(e4m3) DoubleRow matmuls for dots / sum(c^2) / v: K=256 contraction in
    one instruction at 0.5 cycles/row.  wq and wv are pre-scaled by powers of
    two on host (fp8 dynamic range); unscaled on device via exp()'s immediate
    scale and the softmax-weight cast.
  - pass 0 computes dots and sumsq TRANSPOSED ([16-row blocks per n] x tokens)
    by streaming c / c^2 against tiny stationaries, then batched DMA-transposes
    (XBAR) flip them to token-major for the softmax.  No PE transposes, no
    per-(n,tt) stationary reloads.
  - the attention combine runs as [128,1024] n-pair multiplies split across
    DVE and GPSIMD, with the n-reduction as a single strided 2x bf16
    tensor_reduce per token tile.
  - h -> h^T for the output projection via one batched DMA-transpose per token
    tile; output projection accumulates in PSUM with bias applied during the
    PSUM->SBUF copy.
"""

import sys

for _p in ("/opt/trn_rl_repo",):
    if _p not in sys.path:
        sys.path.insert(0, _p)

import numpy as np


B = 8
N = 8          # context slices (softmax axis)
CH = 256       # channels / hidden
H = W = 32
T = H * W      # 1024 spatial tokens per batch
HEADS = 8
HD = 64        # head dim
HS = HEADS * HD  # 512
EPS = 1e-6
NCORES = 8
PT = 128       # partition tile
TT = T // PT   # 8 token tiles
GRP = 4        # token tiles per out-proj batch
TH = T // 2    # token half (512)

USE_FP8 = False             # fp8 v-matmuls measured at rel_err 3.8e-2: the
                            # elementwise quantization noise does not average
                            # out in random-sign dot products.  bf16 passes.
WQ_SCALE = float(2 ** 17)   # folded into exp() immediate scale (fp8 only)
WV_SCALE = float(2 ** 7)    # folded into the softmax-weight cast (fp8 only)
DVE_ADDS = 20               # of the 56 combine adds, how many run on DVE


def _kernel_body(nc, tc, d):
    from contextlib import ExitStack

    from concourse import mybir

    AF = mybir.ActivationFunctionType
    ALU = mybir.AluOpType
    AX = mybir.AxisListType
    PM = mybir.MatmulPerfMode
    f32 = mybir.dt.float32
    bf16 = mybir.dt.bfloat16
    f8 = mybir.dt.float8e4
    cdt = f8 if USE_FP8 else bf16

    with ExitStack() as ctx:
        const = ctx.enter_context(tc.tile_pool(name="const", bufs=1))
        cpool = ctx.enter_context(tc.tile_pool(name="c", bufs=1))
        c2p = ctx.enter_context(tc.tile_pool(name="c2", bufs=3))
        dsbp = ctx.enter_context(tc.tile_pool(name="dsb", bufs=2))
        drawp = ctx.enter_context(tc.tile_pool(name="draw", bufs=1))
        smp = ctx.enter_context(tc.tile_pool(name="sm", bufs=3))
        pap = ctx.enter_context(tc.tile_pool(name="pa", bufs=2))
        hp = ctx.enter_context(tc.tile_pool(name="h", bufs=2))
        htp = ctx.enter_context(tc.tile_pool(name="ht", bufs=2))
        outp = ctx.enter_context(tc.tile_pool(name="o", bufs=2))
        psD = ctx.enter_context(tc.tile_pool(name="psD", bufs=1, space="PSUM"))
        psV = ctx.enter_context(tc.tile_pool(name="psV", bufs=2, space="PSUM"))
        psO = ctx.enter_context(tc.tile_pool(name="psO", bufs=2, space="PSUM"))

        # ---- c loads first (pass-0 waits on c[0]), then weights ----
        c_sb = []
        for n in range(N):
            t = cpool.tile([PT, 2 * T], cdt, tag=f"c{n}", name=f"c{n}")
            eng = nc.sync if n % 2 == 0 else nc.scalar
            eng.dma_start(t[:], d["c"][n, :, :])
            c_sb.append(t)

        wq_sb = const.tile([PT, 64], cdt, tag="wq", name="wq")
        nc.scalar.dma_start(wq_sb[:], d["wq"][:, :])
        on_sb = const.tile([PT, 64], cdt, tag="ones", name="ones")
        nc.scalar.dma_start(on_sb[:], d["ones"][:, :])
        wv_sb = const.tile([PT, 2 * HS], cdt, tag="wv", name="wv")
        nc.scalar.dma_start(wv_sb[:], d["wv"][:, :])
        wo_sb = const.tile([PT, 4 * CH], bf16, tag="wo", name="wo")
        nc.scalar.dma_start(wo_sb[:], d["wo"][:, :])
        bo_sb = const.tile([PT, 2], f32, tag="bo", name="bo")
        nc.scalar.dma_start(bo_sb[:], d["bo"][:, :])
        eps_sb = const.tile([PT, 1], f32, tag="eps", name="eps")
        nc.vector.memset(eps_sb[:], EPS)

        wq3 = wq_sb[:].rearrange("p (k m) -> p k m", k=2)
        on3 = on_sb[:].rearrange("p (k m) -> p k m", k=2)
        wv3 = wv_sb[:].rearrange("p (k m) -> p k m", k=2)
        wo3 = wo_sb[:].rearrange("p (k m) -> p k m", k=4)

        def mm_k256_dr(out_ap, lhsT3, rhs3):
            """contract over 256 channels, full 128-partition output: one
            DoubleRow fp8 matmul or two accumulating bf16 matmuls."""
            if USE_FP8:
                nc.tensor.matmul(out_ap, lhsT3, rhs3, start=True, stop=True,
                                 perf_mode=PM.DoubleRow)
            else:
                for k in range(2):
                    nc.tensor.matmul(out_ap, lhsT3[:, k:k + 1, :],
                                     rhs3[:, k:k + 1, :],
                                     start=(k == 0), stop=(k == 1))

        def mm_k256_nodr(out_ap, lhsT3, rhs3):
            """same contraction but plain (non-DoubleRow) matmuls: needed when
            the output sits at a 32/64 partition offset (PE col tiling is
            incompatible with DoubleRow)."""
            for k in range(2):
                nc.tensor.matmul(out_ap, lhsT3[:, k:k + 1, :],
                                 rhs3[:, k:k + 1, :],
                                 start=(k == 0), stop=(k == 1))

        # ---- pass 0: transposed dots + sumsq, then DMA-transpose ----
        # PSUM matmul outputs may start at partition {0,32,64} only: stack 3
        # context slices per [96, TH] tile (groups (0,1,2) (3,4,5) (6,7,7)).
        # Block for slice n at rows 32*m+j: j<8 -> dots head j / j=0 sumsq.
        GROUPS = [(0, 1, 2), (3, 4, 5), (6, 7, 7)]
        draw_all = []   # per half: [128t, (g3, b4, r96)] bf16
        sraw_all = []
        for half in range(2):
            tsl = slice(half * TH, (half + 1) * TH)
            D_sb = dsbp.tile([96, 3 * TH], bf16, tag="Dsb", name=f"Dsb{half}")
            S_sb = dsbp.tile([96, 3 * TH], bf16, tag="Ssb", name=f"Ssb{half}")
            for g, grp in enumerate(GROUPS):
                D_st = psD.tile([96, TH], f32, tag="D", name=f"Dst{half}{g}")
                for m, n in enumerate(grp):
                    mm_k256_nodr(
                        D_st[32 * m:32 * m + 32, :],
                        wq3,
                        c_sb[n][:].rearrange("p (k t) -> p k t", k=2)[:, :, tsl])
                nc.scalar.activation(
                    D_sb[:, g * TH:(g + 1) * TH], D_st[:], AF.Copy)
                S_st = psD.tile([96, TH], f32, tag="S", name=f"Sst{half}{g}")
                for m, n in enumerate(grp):
                    # squares for this (n, half) only: small c2 ring.
                    # Alternate ACT/DVE (both have capacity; bf16 gets the
                    # DVE 2x packed mode).
                    c2t = c2p.tile([PT, T], cdt, tag="c2",
                                   name=f"c2_{half}_{g}_{m}")
                    c2v = c2t[:].rearrange("p (k t) -> p k t", k=2)
                    csl = c_sb[n][:].rearrange(
                        "p (k t) -> p k t", k=2)[:, :, tsl]
                    if n % 2 == 0:
                        nc.scalar.activation(c2v, csl, AF.Square)
                    else:
                        nc.vector.tensor_mul(c2v, csl, csl)
                    mm_k256_nodr(S_st[32 * m:32 * m + 32, :], on3, c2v)
                nc.scalar.activation(
                    S_sb[:, g * TH:(g + 1) * TH], S_st[:], AF.Copy)

            dr = drawp.tile([PT, 12 * 96], bf16, tag=f"draw{half}",
                            name=f"draw{half}")
            nc.sync.dma_start_transpose(
                dr[:].rearrange("p (j r) -> p j r", j=12), D_sb[:])
            draw_all.append(dr)
            sr = drawp.tile([PT, 12 * 96], bf16, tag=f"sraw{half}",
                            name=f"sraw{half}")
            nc.scalar.dma_start_transpose(
                sr[:].rearrange("p (j r) -> p j r", j=12), S_sb[:])
            sraw_all.append(sr)

        # ---- rms scales, batched per half ----
        # s = 1/sqrt(mean + eps) computed as exp(-0.5 * ln(mean + eps)): ln,
        # exp, square, copy and identity share ONE activation table, so the
        # whole kernel runs without ACT_TABLE_LOAD switches (sqrt does not).
        RUNS = [(0, 0, 3), (1, 3, 3), (2, 6, 2)]   # (group, n0, count)
        s_all, sv_all = [], []
        for half in range(2):
            u = smp.tile([PT, 4 * N], f32, tag="u", name=f"u{half}")
            uv = u[:].rearrange("p (q n) -> p q n", q=4)
            for g, n0, cnt in RUNS:
                srv = sraw_all[half][:].rearrange(
                    "p (g b m j) -> p g b m j", g=3, b=4, m=3) \
                    [:, g:g + 1, :, 0:cnt, 0:1]
                nc.scalar.activation(uv[:, :, n0:n0 + cnt], srv, AF.Ln,
                                     bias=eps_sb[:], scale=1.0 / CH)
            s_h = smp.tile([PT, 4 * N], f32, tag=f"s{half}", name=f"s{half}")
            nc.scalar.activation(s_h[:], u[:], AF.Exp, scale=-0.5)
            sv_h = smp.tile([PT, 4 * N], bf16, tag=f"svh{half}",
                            name=f"sv{half}")
            nc.scalar.activation(
                sv_h[:], s_h[:], AF.Copy,
                scale=(1.0 / WV_SCALE) if USE_FP8 else 1.0)
            s_all.append(s_h)
            sv_all.append(sv_h)

        # ---- pass 1 ----
        add_idx = 0
        add_acc = 0

        def add_engine():
            nonlocal add_idx, add_acc
            add_acc += DVE_ADDS
            add_idx += 1
            if add_acc >= 56:
                add_acc -= 56
                return nc.vector
            return nc.gpsimd

        # ---- pass 1a: softmax for ALL token tiles up front.  It depends
        # only on pass-0 outputs, and hoisting it keeps the in-order DVE/GP
        # queues free of blockers between the heavy combine ops. ----
        av_tiles = []
        for tt in range(TT):
            half, q = tt // 4, tt % 4
            s = s_all[half][:, q * N:(q + 1) * N]
            sv = sv_all[half][:, q * N:(q + 1) * N]

            # Dsc[t, (n,e)] = dots * s   (k-side rms scale) on GPSIMD
            Dsc = smp.tile([PT, N * HEADS], f32, tag="Dsc", name=f"Dsc{tt}")
            for g, n0, cnt in RUNS:
                drv = draw_all[half][:].rearrange(
                    "p (g b m j) -> p g b m j", g=3, b=4, m=3) \
                    [:, g:g + 1, q:q + 1, 0:cnt, 0:HEADS]
                s_bc = s[:, n0:n0 + cnt].rearrange(
                    "p (n o) -> p n o", o=1).broadcast_to([PT, cnt, HEADS])
                nc.gpsimd.tensor_mul(
                    Dsc[:, HEADS * n0:HEADS * (n0 + cnt)].rearrange(
                        "p (n e) -> p n e", n=cnt),
                    drv, s_bc)
            # E2[t, (e,n)] = exp(Dsc / WQ_SCALE)  (reordered write)
            E2 = smp.tile([PT, HEADS * N], bf16, tag="E2", name=f"E2{tt}")
            nc.scalar.activation(
                E2[:].rearrange("p (e n) -> p n e", n=N), Dsc[:], AF.Exp,
                scale=(1.0 / WQ_SCALE) if USE_FP8 else 1.0)
            Z = smp.tile([PT, HEADS], f32, tag="Z", name=f"Z{tt}")
            nc.vector.tensor_reduce(
                Z[:], E2[:].rearrange("p (e n) -> p e n", n=N),
                axis=AX.X, op=ALU.add)
            rZ = smp.tile([PT, HEADS], f32, tag="rZ", name=f"rZ{tt}")
            nc.vector.reciprocal(rZ[:], Z[:])
            # av[t, (e,n)] = E2 * sv (bcast e) * rZ (bcast n)
            av1 = smp.tile([PT, HEADS * N], bf16, tag="av1", name=f"av1{tt}")
            sv_bc = sv.rearrange("p (o n) -> p o n", o=1) \
                      .broadcast_to([PT, HEADS, N])
            nc.vector.tensor_mul(
                av1[:].rearrange("p (e n) -> p e n", n=N),
                E2[:].rearrange("p (e n) -> p e n", n=N), sv_bc)
            av = smp.tile([PT, HEADS * N], bf16, tag=f"av{tt}",
                          name=f"av{tt}")
            rZ_bc = rZ[:].rearrange("p (e o) -> p e o", o=1) \
                         .broadcast_to([PT, HEADS, N])
            nc.gpsimd.tensor_mul(
                av[:].rearrange("p (e n) -> p e n", n=N),
                av1[:].rearrange("p (e n) -> p e n", n=N), rZ_bc)
            av_tiles.append(av)

        # ---- pass 1b: v matmuls + weighted combine per token tile ----
        for tt in range(TT):
            half, q = tt // 4, tt % 4
            av = av_tiles[tt]

            # v matmuls (n-pairs); av-weighting on DVE (the only fast engine
            # that may read PSUM); first-level adds on GPSIMD (SBUF only)
            pall = pap.tile([PT, N * HS], bf16, tag="pall", name=f"pall{tt}")
            qall = pap.tile([PT, 4 * HS], bf16, tag="qall", name=f"qall{tt}")
            for j in range(4):
                vps = psV.tile([PT, 2 * HS], f32, tag="v", name=f"v{tt}_{j}")
                for i in range(2):
                    n = 2 * j + i
                    mm_k256_dr(vps[:, i * HS:(i + 1) * HS],
                               c_sb[n][:].rearrange(
                                   "p (k t) -> p k t",
                                   k=2)[:, :, tt * PT:(tt + 1) * PT],
                               wv3)
                av_pair = av[:].rearrange("p (e n o) -> p n e o", n=N, o=1) \
                    [:, 2 * j:2 * j + 2, :, :] \
                    .broadcast_to([PT, 2, HEADS, HD])
                nc.vector.tensor_mul(
                    pall[:, j * 2 * HS:(j + 1) * 2 * HS].rearrange(
                        "p (n e d) -> p n e d", n=2, e=HEADS),
                    vps[:].rearrange("p (n e d) -> p n e d", n=2, e=HEADS),
                    av_pair)
                add_engine().tensor_add(
                    qall[:, j * HS:(j + 1) * HS],
                    pall[:, j * 2 * HS:j * 2 * HS + HS],
                    pall[:, j * 2 * HS + HS:(j + 1) * 2 * HS])

            # hbig collects h for GRP token tiles; one batched DMA transpose
            if tt % GRP == 0:
                hbig = hp.tile([PT, GRP * HS], bf16, tag="hbig",
                               name=f"hbig{tt // GRP}")
            h2a = hp.tile([PT, HS], bf16, tag="h2a", name=f"h2a{tt}")
            add_engine().tensor_add(h2a[:], qall[:, 0:HS], qall[:, HS:2 * HS])
            h2b = hp.tile([PT, HS], bf16, tag="h2b", name=f"h2b{tt}")
            add_engine().tensor_add(h2b[:], qall[:, 2 * HS:3 * HS],
                                    qall[:, 3 * HS:4 * HS])
            add_engine().tensor_add(hbig[:, q * HS:(q + 1) * HS],
                                    h2a[:], h2b[:])

            if tt % GRP == GRP - 1:
                g = tt // GRP
                ht = htp.tile([PT, GRP * HS], bf16, tag="ht", name=f"ht{g}")
                eng = nc.sync if g % 2 == 0 else nc.scalar
                # ht[p, j, f] = hbig[f, 128*j+p]: j = (tt%GRP)*4 + hs_chunk
                eng.dma_start_transpose(
                    ht[:].rearrange("p (j f) -> p j f", j=4 * GRP), hbig[:])
                for m2 in range(2):
                    o_ps = psO.tile([PT, GRP * PT], f32, tag="o",
                                    name=f"ops{g}_{m2}")
                    for k in range(4):
                        nc.tensor.matmul(
                            o_ps[:],
                            wo3[:, k:k + 1, m2 * PT:(m2 + 1) * PT],
                            ht[:].rearrange("p (t k f) -> p t k f",
                                            t=GRP, k=4)[:, :, k:k + 1, :],
                            start=(k == 0), stop=(k == 3),
                        )
                    o_sb = outp.tile([PT, GRP * PT], f32, tag="o",
                                     name=f"osb{g}_{m2}")
                    nc.scalar.activation(o_sb[:], o_ps[:], AF.Identity,
                                         bias=bo_sb[:, m2:m2 + 1])
                    nc.sync.dma_start(
                        d["out"][m2 * PT:(m2 + 1) * PT,
                                 g * GRP * PT:(g + 1) * GRP * PT],
                        o_sb[:])


def _build_nc():
    import concourse.tile as tile
    from concourse import bacc, mybir

    f32 = mybir.dt.float32
    bf16 = mybir.dt.bfloat16
    cdt = mybir.dt.float8e4 if USE_FP8 else bf16
    nc = bacc.Bacc(
        "TRN2",
        target_bir_lowering=False,
        debug=False,
        enable_asserts=False,
        num_devices=NCORES,
    )
    d = {
        "c": nc.dram_tensor("c", [N, PT, 2 * T], cdt, kind="ExternalInput").ap(),
        "wv": nc.dram_tensor("wv", [PT, 2 * HS], cdt, kind="ExternalInput").ap(),
        "wq": nc.dram_tensor("wq", [PT, 64], cdt, kind="ExternalInput").ap(),
        "ones": nc.dram_tensor("ones", [PT, 64], cdt, kind="ExternalInput").ap(),
        "wo": nc.dram_tensor("wo", [PT, 4 * CH], bf16, kind="ExternalInput").ap(),
        "bo": nc.dram_tensor("bo", [PT, 2], f32, kind="ExternalInput").ap(),
        "out": nc.dram_tensor("out", [CH, T], f32, kind="ExternalOutput").ap(),
    }
    with tile.TileContext(nc) as tc:
        _kernel_body(nc, tc, d)
    nc.compile()
    return nc


_NC_CACHE = None


def _get_nc():
    global _NC_CACHE
    if _NC_CACHE is None:
        _NC_CACHE = _build_nc()
    return _NC_CACHE


def _make_in_maps(q, c, rms_w, emb, w1, b1, w2, b2, w_kv, w_out, b_out):
    q = np.asarray(q).astype(np.int64)
    c = np.asarray(c, dtype=np.float32)
    rms_w = np.asarray(rms_w, dtype=np.float32)
    emb = np.asarray(emb, dtype=np.float32)
    w1 = np.asarray(w1, dtype=np.float32)
    b1 = np.asarray(b1, dtype=np.float32)
    w2 = np.asarray(w2, dtype=np.float32)
    b2 = np.asarray(b2, dtype=np.float32)
    w_kv = np.asarray(w_kv, dtype=np.float32)
    w_out = np.asarray(w_out, dtype=np.float32)
    b_out = np.asarray(b_out, dtype=np.float32)

    # query path (tiny: 8 vectors), exact fp32 math as the reference
    qe = emb[q]                                   # [B, CH]
    x1 = qe @ w1 + b1
    h1 = x1 * (1.0 / (1.0 + np.exp(-x1)))         # silu
    qh = (h1 @ w2 + b2).reshape(B, HEADS, HD)

    wkv3 = w_kv.reshape(CH, HEADS, 2 * HD)
    w_k = wkv3[:, :, :HD]                         # [CH, HEADS, HD]
    w_v = wkv3[:, :, HD:]
    wv = (rms_w[:, None, None] * w_v).reshape(CH, HS)
    scale = float(HD) ** -0.5
    # wq[b, ch, e] = rms_w[ch] * scale * sum_d w_k[ch, e, d] * qh[b, e, d]
    wq_all = np.einsum("ced,bed->bce", w_k, qh).astype(np.float32)
    wq_all = wq_all * (scale * rms_w[None, :, None])

    import ml_dtypes
    bf = ml_dtypes.bfloat16
    cnp = ml_dtypes.float8_e4m3 if USE_FP8 else bf

    def chsplit(x):
        # [CH, M] -> [128, 2, M] -> [128, 2*M]
        m = x.shape[1]
        return np.ascontiguousarray(
            x.reshape(2, PT, m).transpose(1, 0, 2).reshape(PT, 2 * m))

    wvs = wv * (WV_SCALE if USE_FP8 else 1.0)
    wq_pad = np.zeros((B, CH, 32), dtype=np.float32)
    wq_pad[:, :, :HEADS] = wq_all * (WQ_SCALE if USE_FP8 else 1.0)
    ones_pad = np.zeros((CH, 32), dtype=np.float32)
    ones_pad[:, 0] = 1.0

    shared = {
        "wv": chsplit(wvs).astype(cnp),
        "ones": chsplit(ones_pad).astype(cnp),
        "wo": np.ascontiguousarray(
            w_out.reshape(4, PT, CH).transpose(1, 0, 2).reshape(PT, 4 * CH)
        ).astype(bf),
        "bo": np.ascontiguousarray(
            b_out.reshape(2, PT).T, dtype=np.float32),
    }
    in_maps = []
    for b in range(B):
        m = dict(shared)
        cb = c[b].reshape(N, 2, PT, T).transpose(0, 2, 1, 3).reshape(N, PT, 2 * T)
        m["c"] = np.ascontiguousarray(cb).astype(cnp)
        m["wq"] = chsplit(wq_pad[b]).astype(cnp)
        in_maps.append(m)
    return in_maps


def _run(in_maps, **kwargs):
    from concourse import bass_utils

    nc = _get_nc()
    return bass_utils.run_bass_kernel_spmd(
        nc, in_maps, core_ids=list(range(NCORES)), **kwargs)


def kernel(q, c, rms_w, emb, w1, b1, w2, b2, w_kv, w_out, b_out):
    in_maps = _make_in_maps(q, c, rms_w, emb, w1, b1, w2, b2, w_kv, w_out,
                            b_out)
    res = _run(in_maps)
    outs = [np.asarray(res.results[b]["out"]).reshape(CH, H, W)
            for b in range(B)]
    return np.stack(outs, axis=0)


# revision 33
# speedup vs baseline: 1.1899x; 1.1899x over previous
"""Trainium2 Bass kernel for nn_Attention_24043226923261.

Per-pixel cross-attention: RMSNorm(c) -> kv proj -> softmax over N=8 context
slices with a query shared across the 32x32 spatial grid -> out proj.

Sharding: data-parallel over B=8 across the 8 NeuronCores (core b owns batch
b). Zero collectives.

Key algebraic restructuring (host-side weight folding, exact math):
  - query path qh = silu(emb[q]@w1+b1)@w2+b2 is a [8,512] tensor; dots =
    qh . (c_norm @ w_k) = c_norm @ (w_k @ qh^T), so fold qh, attn_scale and
    rms_w into a per-core [256,8] matrix wq.  k is never materialized and the
    kv projection halves to v-only.
  - rms_w folds into wv/wq; the per-token rsqrt(mean(c^2)) scale s_n[t] is
    applied on device: on the k side inside exp() via the activation's
    per-partition scale, on the v side by folding into the softmax weights.
  - out proj is computed transposed (out^T = w_out^T @ h^T) so the result
    lands channel-major [256, H*W], which is exactly the required output
    layout.
"""

import sys

for _p in ("/opt/trn_rl_repo",):
    if _p not in sys.path:
        sys.path.insert(0, _p)

import numpy as np


B = 8
N = 8          # context slices (softmax axis)
CH = 256       # channels / hidden
H = W = 32
T = H * W      # 1024 spatial tokens per batch
HEADS = 8
HD = 64        # head dim
HS = HEADS * HD  # 512
EPS = 1e-6
NCORES = 8
PT = 128       # partition tile
TT = T // PT   # 8 token tiles
KCH = CH // PT  # 2 contraction chunks over channels
KHS = HS // PT  # 4 contraction chunks over (head, d)
GRP = 4        # token tiles per out-proj batch


def _kernel_body(nc, tc, d):
    from contextlib import ExitStack

    from concourse import mybir

    AF = mybir.ActivationFunctionType
    ALU = mybir.AluOpType
    AX = mybir.AxisListType
    f32 = mybir.dt.float32
    bf16 = mybir.dt.bfloat16

    with ExitStack() as ctx:
        const = ctx.enter_context(tc.tile_pool(name="const", bufs=1))
        cpool = ctx.enter_context(tc.tile_pool(name="c", bufs=1))
        c2p = ctx.enter_context(tc.tile_pool(name="c2", bufs=5))
        sp = ctx.enter_context(tc.tile_pool(name="s", bufs=1))
        ep = ctx.enter_context(tc.tile_pool(name="e", bufs=2))
        avp = ctx.enter_context(tc.tile_pool(name="av", bufs=3))
        hp = ctx.enter_context(tc.tile_pool(name="h", bufs=2))
        prodp = ctx.enter_context(tc.tile_pool(name="prod", bufs=3))
        htp = ctx.enter_context(tc.tile_pool(name="ht", bufs=2))
        outp = ctx.enter_context(tc.tile_pool(name="o", bufs=2))
        psD = ctx.enter_context(tc.tile_pool(name="psD", bufs=1, space="PSUM"))
        psV = ctx.enter_context(tc.tile_pool(name="psV", bufs=3, space="PSUM"))
        psT = ctx.enter_context(tc.tile_pool(name="psT", bufs=2, space="PSUM"))
        psO = ctx.enter_context(tc.tile_pool(name="psO", bufs=2, space="PSUM"))

        # ---- constants + c loads, ordered so c[0] lands early ----
        wq_sb = []
        invc_sb = []
        for k in range(KCH):
            t = const.tile([PT, HEADS], bf16, tag=f"wq{k}", name=f"wq{k}")
            nc.sync.dma_start(t[:], d["wq"][k * PT:(k + 1) * PT, :])
            wq_sb.append(t)
            t = const.tile([PT, 1], bf16, tag=f"invc{k}", name=f"invc{k}")
            nc.sync.dma_start(t[:], d["invc"][k * PT:(k + 1) * PT, :])
            invc_sb.append(t)
        eps_sb = const.tile([PT, 1], f32, tag="eps", name="eps")
        nc.vector.memset(eps_sb[:], EPS)

        c_sb = {}
        for k in range(KCH):
            t = cpool.tile([PT, T], bf16, tag=f"c0_{k}", name=f"c0_{k}")
            nc.sync.dma_start(t[:], d["c"][0, k * PT:(k + 1) * PT, :])
            c_sb[0, k] = t

        for n in range(1, N):
            # GPSIMD (idle through pass 0) carries two early slices on its
            # own DMA rings to parallelize the load ramp
            eng = nc.gpsimd if n in (1, 2) else nc.sync
            for k in range(KCH):
                t = cpool.tile([PT, T], bf16, tag=f"c{n}_{k}", name=f"c{n}_{k}")
                eng.dma_start(t[:], d["c"][n, k * PT:(k + 1) * PT, :])
                c_sb[n, k] = t

        # pass-1 weights load after all of c: they are not needed until
        # ~40us in, and this keeps the sync queue clear for pass-0 data
        wv_sb = []
        for k in range(KCH):
            t = const.tile([PT, HS], bf16, tag=f"wv{k}", name=f"wv{k}")
            nc.sync.dma_start(t[:], d["wv"][k * PT:(k + 1) * PT, :])
            wv_sb.append(t)
        wo_sb = []
        for k in range(KHS):
            t = const.tile([PT, CH], bf16, tag=f"wo{k}", name=f"wo{k}")
            nc.sync.dma_start(t[:], d["wo"][k * PT:(k + 1) * PT, :])
            wo_sb.append(t)
        bo_sb = []
        for m in range(CH // PT):
            t = const.tile([PT, 1], f32, tag=f"bo{m}", name=f"bo{m}")
            nc.sync.dma_start(t[:], d["bo"][m * PT:(m + 1) * PT, :])
            bo_sb.append(t)
        eye_sb = const.tile([PT, PT], bf16, tag="eye", name="eye")
        nc.sync.dma_start(eye_sb[:], d["eye"][:, :])

        D_ps = psD.tile([PT, TT * HEADS * N], f32, name="D")
        Dv = D_ps[:].rearrange("p (a e n) -> p a e n", a=TT, n=N)
        # s_all[p, (tt, n)] = rsqrt(mean_n(c^2) + eps); sq_all holds sqrt
        sq_all = sp.tile([PT, TT * N], f32, tag="sq", name="sq_all")
        sqv = sq_all[:].rearrange("p (a n) -> p a n", n=N)
        s_all = sp.tile([PT, TT * N], f32, tag="s", name="s_all")
        sv = s_all[:].rearrange("p (a n) -> p a n", n=N)

        # ---- pass 0: per context slice n: squares, mean, s, dots ----
        for n in range(N):
            for tt in range(TT):
                for k in range(KCH):
                    nc.tensor.matmul(
                        Dv[:, tt, :, n],
                        c_sb[n, k][:, tt * PT:(tt + 1) * PT],
                        wq_sb[k][:],
                        start=(k == 0), stop=(k == KCH - 1),
                    )
            c2 = [c2p.tile([PT, T], bf16, tag="c2", name=f"c2_{n}_{_k}") for _k in range(KCH)]
            for k in range(KCH):
                if n % 2 == 0:
                    nc.scalar.activation(c2[k][:], c_sb[n, k][:], AF.Square)
                else:
                    nc.vector.tensor_mul(c2[k][:], c_sb[n, k][:],
                                         c_sb[n, k][:])
            mean_ps = psO.tile([PT, TT], f32, tag="o", name=f"mean{n}")
            for tt in range(TT):
                for k in range(KCH):
                    nc.tensor.matmul(
                        mean_ps[:, tt:tt + 1],
                        c2[k][:, tt * PT:(tt + 1) * PT],
                        invc_sb[k][:],
                        start=(k == 0), stop=(k == KCH - 1),
                    )
            nc.scalar.activation(sqv[:, :, n], mean_ps[:], AF.Sqrt,
                                 bias=eps_sb[:])
        nc.vector.reciprocal(s_all[:], sq_all[:])

        # ---- pass 1a: softmax for ALL token tiles up front (depends only
        # on pass-0); keeps the in-order DVE/GP queues free of blockers
        # between the heavy combine ops ----
        av_tiles = []
        for tt in range(TT):
            s_bc = sv[:, tt:tt + 1, :].broadcast_to([PT, HEADS, N])
            # Dsc = dots * s (k-side rms scale), then one exp for all (e, n)
            Dsc = ep.tile([PT, HEADS * N], f32, tag="Dsc", name=f"Dsc{tt}")
            nc.vector.tensor_mul(
                Dsc[:].rearrange("p (e n) -> p e n", n=N),
                Dv[:, tt, :, :], s_bc)
            E = ep.tile([PT, HEADS * N], f32, tag="E", name=f"E{tt}")
            Ev = E[:].rearrange("p (e n) -> p e n", n=N)
            nc.scalar.activation(E[:], Dsc[:], AF.Exp)
            Z = ep.tile([PT, HEADS], f32, tag="Z", name=f"Z{tt}")
            nc.vector.tensor_reduce(Z[:], Ev, axis=AX.X, op=ALU.add)
            rZ = ep.tile([PT, HEADS], f32, tag="rZ", name=f"rZ{tt}")
            nc.vector.reciprocal(rZ[:], Z[:])
            # attnv[p, e, n] = E * (1/Z) [bcast over n] * s [bcast over e]
            rZ_bc = rZ[:].rearrange("p (e one) -> p e one", one=1) \
                         .broadcast_to([PT, HEADS, N])
            av_all = avp.tile([PT, HEADS * N], f32, tag=f"av{tt}",
                              name=f"av{tt}")
            avv = av_all[:].rearrange("p (e n) -> p e n", n=N)
            nc.vector.tensor_mul(avv, Ev, rZ_bc)
            nc.gpsimd.tensor_mul(avv, avv, s_bc)
            av_tiles.append(av_all)

        # ---- pass 1b: per token tile: v matmul, h, transpose, out ----
        ht_sb = None
        for tt in range(TT):
            if tt % GRP == 0:
                ht_sb = [htp.tile([PT, GRP * PT], bf16, tag=f"ht{k}", name=f"ht{k}_{tt}")
                         for k in range(KHS)]
            avv = av_tiles[tt][:].rearrange("p (e n) -> p e n", n=N)

            h = hp.tile([PT, HS], bf16, tag="h", name=f"h{tt}")
            for n in range(N):
                v_ps = psV.tile([PT, HS], f32, tag="v", name=f"v{tt}_{n}")
                for k in range(KCH):
                    nc.tensor.matmul(
                        v_ps[:],
                        c_sb[n, k][:, tt * PT:(tt + 1) * PT],
                        wv_sb[k][:],
                        start=(k == 0), stop=(k == KCH - 1),
                    )
                av_b = avv[:, :, n:n + 1].broadcast_to([PT, HEADS, HD])
                tgt = h if n == 0 else prodp.tile([PT, HS], bf16, tag="prod", name=f"prod{tt}_{n}")
                nc.vector.tensor_mul(
                    tgt[:].rearrange("p (e d) -> p e d", d=HD),
                    v_ps[:].rearrange("p (e d) -> p e d", d=HD),
                    av_b,
                )
                if n > 0:
                    # last tile: keep the serial chain on the fast engine —
                    # there is no following work to hide the slow Q7 adds
                    if tt % GRP == GRP - 1:
                        eng = nc.vector
                    else:
                        eng = nc.gpsimd if n % 2 == 0 else nc.vector
                    eng.tensor_add(h[:], h[:], tgt[:])

            for m in range(KHS):
                tr = psT.tile([PT, PT], bf16, tag="tr", name=f"tr{tt}_{m}")
                nc.tensor.transpose(tr[:], h[:, m * PT:(m + 1) * PT], eye_sb[:])
                nc.scalar.copy(
                    ht_sb[m][:, (tt % GRP) * PT:(tt % GRP + 1) * PT], tr[:])

            if tt % GRP == GRP - 1:
                g = tt // GRP
                for m2 in range(CH // PT):
                    o_ps = psO.tile([PT, GRP * PT], f32, tag="o", name=f"ops{tt}_{m2}")
                    for k in range(KHS):
                        nc.tensor.matmul(
                            o_ps[:],
                            wo_sb[k][:, m2 * PT:(m2 + 1) * PT],
                            ht_sb[k][:],
                            start=(k == 0), stop=(k == KHS - 1),
                        )
                    o_sb = outp.tile([PT, GRP * PT], f32, tag="o", name=f"osb{tt}_{m2}")
                    nc.scalar.activation(o_sb[:], o_ps[:], AF.Identity,
                                         bias=bo_sb[m2][:])
                    nc.sync.dma_start(
                        d["out"][m2 * PT:(m2 + 1) * PT,
                                 g * GRP * PT:(g + 1) * GRP * PT],
                        o_sb[:])


def _build_nc():
    import concourse.tile as tile
    from concourse import bacc, mybir

    f32 = mybir.dt.float32
    bf16 = mybir.dt.bfloat16
    nc = bacc.Bacc(
        "TRN2",
        target_bir_lowering=False,
        debug=False,
        enable_asserts=False,
        num_devices=NCORES,
    )
    d = {
        "c": nc.dram_tensor("c", [N, CH, T], bf16, kind="ExternalInput").ap(),
        "wv": nc.dram_tensor("wv", [CH, HS], bf16, kind="ExternalInput").ap(),
        "wq": nc.dram_tensor("wq", [CH, HEADS], bf16, kind="ExternalInput").ap(),
        "wo": nc.dram_tensor("wo", [HS, CH], bf16, kind="ExternalInput").ap(),
        "bo": nc.dram_tensor("bo", [CH, 1], f32, kind="ExternalInput").ap(),
        "invc": nc.dram_tensor("invc", [CH, 1], bf16,
                               kind="ExternalInput").ap(),
        "eye": nc.dram_tensor("eye", [PT, PT], bf16, kind="ExternalInput").ap(),
        "eye32": nc.dram_tensor("eye32", [PT, PT], f32,
                                kind="ExternalInput").ap(),
        "out": nc.dram_tensor("out", [CH, T], f32, kind="ExternalOutput").ap(),
    }
    with tile.TileContext(nc) as tc:
        _kernel_body(nc, tc, d)
    nc.compile()
    return nc


_NC_CACHE = None


def _get_nc():
    global _NC_CACHE
    if _NC_CACHE is None:
        _NC_CACHE = _build_nc()
    return _NC_CACHE


def _make_in_maps(q, c, rms_w, emb, w1, b1, w2, b2, w_kv, w_out, b_out):
    q = np.asarray(q).astype(np.int64)
    c = np.asarray(c, dtype=np.float32)
    rms_w = np.asarray(rms_w, dtype=np.float32)
    emb = np.asarray(emb, dtype=np.float32)
    w1 = np.asarray(w1, dtype=np.float32)
    b1 = np.asarray(b1, dtype=np.float32)
    w2 = np.asarray(w2, dtype=np.float32)
    b2 = np.asarray(b2, dtype=np.float32)
    w_kv = np.asarray(w_kv, dtype=np.float32)
    w_out = np.asarray(w_out, dtype=np.float32)
    b_out = np.asarray(b_out, dtype=np.float32)

    # query path (tiny: 8 vectors), exact fp32 math as the reference
    qe = emb[q]                                   # [B, CH]
    x1 = qe @ w1 + b1
    h1 = x1 * (1.0 / (1.0 + np.exp(-x1)))         # silu
    qh = (h1 @ w2 + b2).reshape(B, HEADS, HD)

    wkv3 = w_kv.reshape(CH, HEADS, 2 * HD)
    w_k = wkv3[:, :, :HD]                         # [CH, HEADS, HD]
    w_v = wkv3[:, :, HD:]
    wv = np.ascontiguousarray(
        (rms_w[:, None, None] * w_v).reshape(CH, HS), dtype=np.float32)
    scale = float(HD) ** -0.5
    # wq[b, ch, e] = rms_w[ch] * scale * sum_d w_k[ch, e, d] * qh[b, e, d]
    wq_all = np.einsum("ced,bed->bce", w_k, qh).astype(np.float32)
    wq_all = wq_all * (scale * rms_w[None, :, None])

    import ml_dtypes
    bf = ml_dtypes.bfloat16
    shared = {
        "wv": wv.astype(bf),
        "wo": np.ascontiguousarray(w_out).astype(bf),
        "bo": np.ascontiguousarray(b_out.reshape(CH, 1), dtype=np.float32),
        "invc": np.full((CH, 1), 1.0 / CH, dtype=np.float32).astype(bf),
        "eye": np.eye(PT, dtype=np.float32).astype(bf),
        "eye32": np.eye(PT, dtype=np.float32),
    }
    in_maps = []
    for b in range(B):
        m = dict(shared)
        m["c"] = np.ascontiguousarray(c[b].reshape(N, CH, T)).astype(bf)
        m["wq"] = np.ascontiguousarray(wq_all[b]).astype(bf)
        in_maps.append(m)
    return in_maps


def _run(in_maps, **kwargs):
    from concourse import bass_utils

    nc = _get_nc()
    return bass_utils.run_bass_kernel_spmd(
        nc, in_maps, core_ids=list(range(NCORES)), **kwargs)


def kernel(q, c, rms_w, emb, w1, b1, w2, b2, w_kv, w_out, b_out):
    in_maps = _make_in_maps(q, c, rms_w, emb, w1, b1, w2, b2, w_kv, w_out,
                            b_out)
    res = _run(in_maps)
    outs = [np.asarray(res.results[b]["out"]).reshape(CH, H, W)
            for b in range(B)]
    return np.stack(outs, axis=0)

